# revision 11
# baseline (speedup 1.0000x reference)
"""DINOv2 LoRA featurizer histogram-binning kernel for TRN2 (8 NeuronCores).

Reference computation (per sample):
  x: [37, 37, 384] -> bx = x^T [384, 37, 37]
  pool0 = bx, pool1 = AvgPool2d(3, stride 1, pad 1, count_include_pad=False)
  17 bins = border-clamped shifts of pool0 (9 bins, offsets +-1) and
  pool1 (8 bins, offsets +-3); bins 17..28 of 29 are zero.
  out = [29*384, 37, 37] with channel c = bin*384 + feature.

Sharding: pure data parallel, sample b -> core b (B == 8 == n_cores).

The kernel is store-bandwidth bound (17 bins x 2.1 MB per core). Writing
fp16 (rel err ~4e-4, far inside the 2e-2 gate) halves the store stream;
only the 17 real bins live in DRAM and the 12 zero bins are assembled
host-side as zeros.

Device strategy (per core), built so the store-DMA stream (~18 MB fp16)
is the only critical path:
  - channels on partitions (3 tiles of 128), spatial flattened in free dim
  - x is uploaded as fp16 and DMA'd straight into the dx=0 pool0 plane,
    so k=0 bins are exactly fp16(x); loads ride the scalar-engine HWDGE
    ring so they are off the store rings entirely
  - for each (pool k, dx) a column-pre-shifted, row-replicated-padded plane
    R[k][dxi][t]; every bin is then a CONTIGUOUS row-window of one plane.
    Stores are grouped per (ctile, dy) — 3 adjacent bins per DMA so each
    partition writes an 8 KiB contiguous DRAM run (measured fastest;
    both fewer/bigger and more/smaller DMAs measured slower)
  - stores alternate between the sync and scalar HWDGE rings to keep more
    descriptors in flight (the gpsimd SWDGE ring is pathologically slow
    to drain - do not use it)
  - dx!=0 planes are flat-shifted whole-plane fp16 copies (column shift ==
    element offset in the flat layout; row-boundary wrap errors land
    exactly in the clamped columns, fixed by strided edge-column copies)
  - pool1 separable 3x3 SUM runs in fp16 (2x DVE rate; |sums| <= ~20 so
    fp16 range is safe); count_include_pad=False normalization factorizes
    (cnt = rowcnt[i]*colcnt[j], each in {2,3}), so ONE tensor_scalar mul
    by 1/9 normalizes the plane and four tiny edge-region muls by 1.5 fix
    the border rows/columns
  - DVE does all element work (fp16 copies measured ~0.3 ns/elem; the Act
    engine has no fp16 speedup and mid-chain cross-engine handoffs cost
    more than they save)
  - no stride-0 (broadcast) APs, no GpSimd data ops (both measured slow)
"""

import numpy as np

B = 8
W = 37          # spatial side
WW = W * W      # 1369
D = 384
P = 128
ST = D // P     # 3 channel tiles of 128
NBINS = 29
NWR = 17        # bins actually written (the rest are zero)
PAD0, PAD1 = 1, 3
R0ROWS = W + 2 * PAD0             # 39
R1ROWS = W + 2 * PAD1             # 43
R0F = R0ROWS * W                  # 1443 flat elems per plane
R1F = R1ROWS * W                  # 1591

_CACHE = {}


def _build_nc():
    import concourse.bass as bass  # noqa: F401
    import concourse.tile as tile
    from concourse import bacc, mybir
    from contextlib import ExitStack

    f16 = mybir.dt.float16
    nc = bacc.Bacc("TRN2", target_bir_lowering=False, debug=False)

    xt = nc.declare_dram_parameter("xt", [ST, P, WW], f16, isOutput=False)
    out = nc.declare_dram_parameter("out", [ST, P, NWR, WW], f16, isOutput=True)

    with tile.TileContext(nc) as tc, ExitStack() as ctx:
        perm = ctx.enter_context(tc.tile_pool(name="perm", bufs=1))
        tmp = ctx.enter_context(tc.tile_pool(name="tmp", bufs=2))
        # G1 gets one buffer per ctile: with only 2, G1(t2) would reuse
        # G1(t0)'s buffer and stall on k1(t0)'s DMA *completion*
        g1p = ctx.enter_context(tc.tile_pool(name="g1p", bufs=3))

        # R0: [dxi, t, 39, 37] (pad 1, dx in {-1,0,+1})
        # R1: [dxi, t, 43, 37] (pad 3, dx in {-3,0,+3})
        R0 = perm.tile([P, 3, ST, R0ROWS, W], f16, name="R0")
        R1 = perm.tile([P, 3, ST, R1ROWS, W], f16, name="R1")

        # ---- load fp16 x into the dx=0 plane centers ----
        for t in range(ST):
            nc.sync.dma_start(
                R0[:, 1, t, PAD0 : PAD0 + W, :].rearrange("p a b -> p (a b)"),
                xt.ap()[t],
            )

        for t in range(ST):
            # pad rows of the dx=0 plane (replicate first/last x row)
            nc.vector.tensor_copy(R0[:, 1, t, 0, :], R0[:, 1, t, 1, :])
            nc.vector.tensor_copy(R0[:, 1, t, R0ROWS - 1, :], R0[:, 1, t, R0ROWS - 2, :])

            # ---- dx=+-1 planes: flat-shifted whole-plane copies + col fix
            # (pad rows propagate; wrap errors land in the clamped column) ----
            p0f = R0[:, 1, t].rearrange("p a b -> p (a b)")
            f = R0[:, 0, t].rearrange("p a b -> p (a b)")
            nc.vector.tensor_copy(f[:, 1:R0F], p0f[:, 0 : R0F - 1])
            nc.vector.tensor_copy(R0[:, 0, t, :, 0], R0[:, 1, t, :, 0])
            f = R0[:, 2, t].rearrange("p a b -> p (a b)")
            nc.vector.tensor_copy(f[:, 0 : R0F - 1], p0f[:, 1:R0F])
            nc.vector.tensor_copy(R0[:, 2, t, :, W - 1], R0[:, 1, t, :, W - 1])

            # ---- k=0 stores for this ctile (one DMA per dy group) ----
            for r_i, dy in enumerate((-1, 0, 1)):
                src = R0[:, :, t, PAD0 + dy : PAD0 + dy + W, :].rearrange(
                    "p x a b -> p x (a b)"
                )
                nc.sync.dma_start(out.ap()[t][:, 3 * r_i : 3 * r_i + 3, :], src)

        for t in range(ST):
            Xc = R0[:, 1, t, PAD0 : PAD0 + W, :].rearrange("p a b -> p (a b)")
            Xc3 = R0[:, 1, t, PAD0 : PAD0 + W, :]

            # ---- column pass (fp16 sums):
            # T[i,j] = sum_dx X[i, j+dx] (zero outside) ----
            T = tmp.tile([P, WW], f16, name="T", tag="T")
            T3 = T.rearrange("p (a b) -> p a b", a=W, b=W)
            nc.vector.tensor_add(T[:, 0 : WW - 1], Xc[:, 0 : WW - 1], Xc[:, 1:WW])
            nc.vector.tensor_copy(T[:, WW - 1 : WW], Xc[:, WW - 1 : WW])
            nc.vector.tensor_add(T[:, 1:WW], T[:, 1:WW], Xc[:, 0 : WW - 1])
            nc.vector.tensor_add(T3[:, :, 0], Xc3[:, :, 0], Xc3[:, :, 1])
            nc.vector.tensor_add(T3[:, :, W - 1], Xc3[:, :, W - 2], Xc3[:, :, W - 1])

            # ---- row pass into a padded fp16 SUM plane ----
            Sp = tmp.tile([P, R1F], f16, name="Sp", tag="Sp")
            c0 = PAD1 * W                        # 111: center start
            nW = WW - W
            nc.vector.tensor_add(Sp[:, c0 : c0 + nW], T[:, 0:nW], T[:, W:WW])
            nc.vector.tensor_copy(Sp[:, c0 + nW : c0 + WW], T[:, nW:WW])
            nc.vector.tensor_add(Sp[:, c0 + W : c0 + WW], Sp[:, c0 + W : c0 + WW], T[:, 0:nW])
            # pad rows: replicate first/last center row (contiguous copies)
            for i in range(PAD1):
                nc.vector.tensor_copy(Sp[:, i * W : (i + 1) * W], Sp[:, c0 : c0 + W])
                nc.vector.tensor_copy(
                    Sp[:, (PAD1 + W + i) * W : (PAD1 + W + i + 1) * W],
                    Sp[:, (PAD1 + W - 1) * W : (PAD1 + W) * W],
                )

            # ---- normalize in ONE pass: interior count is 9; border
            # rows/cols have count 2 (not 3) per axis -> x1.5 ----
            Pc = R1[:, 1, t]
            Pcf = Pc.rearrange("p a b -> p (a b)")
            nc.vector.tensor_scalar_mul(Pcf[:, :], Sp[:, :], 1.0 / 9.0)
            nc.vector.tensor_scalar_mul(Pcf[:, 0 : c0 + W], Pcf[:, 0 : c0 + W], 1.5)
            nc.vector.tensor_scalar_mul(
                Pcf[:, R1F - c0 - W : R1F], Pcf[:, R1F - c0 - W : R1F], 1.5
            )
            nc.vector.tensor_scalar_mul(Pc[:, :, 0], Pc[:, :, 0], 1.5)
            nc.vector.tensor_scalar_mul(Pc[:, :, W - 1], Pc[:, :, W - 1], 1.5)

            # ---- dx=+-3 planes: flat-shifted fp16 copies + edge-col fixes ----
            f = R1[:, 0, t].rearrange("p a b -> p (a b)")
            nc.vector.tensor_copy(f[:, 3:R1F], Pcf[:, 0 : R1F - 3])
            f = R1[:, 2, t].rearrange("p a b -> p (a b)")
            nc.vector.tensor_copy(f[:, 0 : R1F - 3], Pcf[:, 3:R1F])
            for c in range(PAD1):
                nc.vector.tensor_copy(R1[:, 0, t, :, c], Pc[:, :, 0])
                nc.vector.tensor_copy(R1[:, 2, t, :, W - 1 - c], Pc[:, :, W - 1])

            # ---- k=1 stores: stage all 8 bins into ONE contiguous SBUF
            # block so the store DMA reads 21.9 KiB rows per partition
            # (fp16 2738B rows were measured row-rate limited ~340 GB/s) ----
            G1 = g1p.tile([P, 8, WW], f16, name="G1", tag="G1")
            pf = [R1[:, dxp, t].rearrange("p a b -> p (a b)") for dxp in range(3)]
            for s, (dy, dxp) in enumerate(
                ((-3, 0), (-3, 1), (-3, 2), (0, 0), (0, 2), (3, 0), (3, 1), (3, 2))
            ):
                lo = (PAD1 + dy) * W
                nc.vector.tensor_copy(G1[:, s, :], pf[dxp][:, lo : lo + WW])
            nc.sync.dma_start(out.ap()[t][:, 9:17, :], G1[:, :, :])

    nc.compile()
    return nc


def get_nc():
    if "nc" not in _CACHE:
        _CACHE["nc"] = _build_nc()
    return _CACHE["nc"]


def make_in_maps(x: np.ndarray):
    x = np.ascontiguousarray(x, dtype=np.float32)
    assert x.shape == (B, W, W, D), x.shape
    maps = []
    for b in range(B):
        xtr = x[b].transpose(2, 0, 1).reshape(ST, P, WW).astype(np.float16)
        maps.append({"xt": np.ascontiguousarray(xtr)})
    return maps


def decode_core(o, out):
    """Scatter the device 'out' tensor of one sample into out [NBINS*D, W, W]."""
    ob = np.asarray(o, dtype=np.float32)
    ob = ob.reshape(D, NWR, W, W).transpose(1, 0, 2, 3)
    out[: NWR * D] = ob.reshape(NWR * D, W, W)


def run(x: np.ndarray, **kw):
    from concourse.bass_utils import run_bass_kernel_spmd

    nc = get_nc()
    res = run_bass_kernel_spmd(nc, make_in_maps(x), core_ids=list(range(B)), **kw)
    outs = np.zeros((B, NBINS * D, W, W), np.float32)
    for b in range(B):
        decode_core(res.results[b]["out"], outs[b])
    return outs, res


def kernel(x: np.ndarray) -> np.ndarray:
    outs, _ = run(x)
    return outs


# revision 12
# speedup vs baseline: 1.1370x; 1.1370x over previous
"""DINOv2 LoRA featurizer histogram-binning kernel for TRN2 (8 NeuronCores).

Reference computation (per sample):
  x: [37, 37, 384] -> bx = x^T [384, 37, 37]
  pool0 = bx, pool1 = AvgPool2d(3, stride 1, pad 1, count_include_pad=False)
  17 bins = border-clamped shifts of pool0 (9 bins, offsets +-1) and
  pool1 (8 bins, offsets +-3); bins 17..28 of 29 are zero.
  out = [29*384, 37, 37] with channel c = bin*384 + feature.

Sharding: pure data parallel, sample b -> core b (B == 8 == n_cores).

The kernel is store-bandwidth bound (17 bins x 2.1 MB per core). Writing
fp16 (rel err ~4e-4, far inside the 2e-2 gate) halves the store stream;
only the 17 real bins live in DRAM and the 12 zero bins are assembled
host-side as zeros.

Device strategy (per core), built so the store-DMA stream (~18 MB fp16)
is the only critical path:
  - channels on partitions (3 tiles of 128), spatial flattened in free dim
  - x is uploaded as fp16 and DMA'd straight into the dx=0 pool0 plane,
    so k=0 bins are exactly fp16(x); loads ride the scalar-engine HWDGE
    ring so they are off the store rings entirely
  - for each (pool k, dx) a column-pre-shifted, row-replicated-padded plane
    R[k][dxi][t]; every bin is then a CONTIGUOUS row-window of one plane.
    Stores are grouped per (ctile, dy) — 3 adjacent bins per DMA so each
    partition writes an 8 KiB contiguous DRAM run (measured fastest;
    both fewer/bigger and more/smaller DMAs measured slower)
  - stores alternate between the sync and scalar HWDGE rings to keep more
    descriptors in flight (the gpsimd SWDGE ring is pathologically slow
    to drain - do not use it)
  - dx!=0 planes are flat-shifted whole-plane fp16 copies (column shift ==
    element offset in the flat layout; row-boundary wrap errors land
    exactly in the clamped columns, fixed by strided edge-column copies)
  - pool1 separable 3x3 SUM runs in fp16 (2x DVE rate; |sums| <= ~20 so
    fp16 range is safe); count_include_pad=False normalization factorizes
    (cnt = rowcnt[i]*colcnt[j], each in {2,3}), so ONE tensor_scalar mul
    by 1/9 normalizes the plane and four tiny edge-region muls by 1.5 fix
    the border rows/columns
  - DVE does all element work (fp16 copies measured ~0.3 ns/elem; the Act
    engine has no fp16 speedup and mid-chain cross-engine handoffs cost
    more than they save)
  - no stride-0 (broadcast) APs, no GpSimd data ops (both measured slow)
"""

import numpy as np

B = 8
W = 37          # spatial side
WW = W * W      # 1369
D = 384
P = 128
ST = D // P     # 3 channel tiles of 128
NBINS = 29
NWR = 17        # bins actually written (the rest are zero)
PAD0, PAD1 = 1, 3
R0ROWS = W + 2 * PAD0             # 39
R1ROWS = W + 2 * PAD1             # 43
R0F = R0ROWS * W                  # 1443 flat elems per plane
R1F = R1ROWS * W                  # 1591

_CACHE = {}


def _build_nc():
    import concourse.bass as bass  # noqa: F401
    import concourse.tile as tile
    from concourse import bacc, mybir
    from contextlib import ExitStack

    f16 = mybir.dt.float16
    nc = bacc.Bacc("TRN2", target_bir_lowering=False, debug=False)

    xt = nc.declare_dram_parameter("xt", [ST, P, WW], f16, isOutput=False)
    out = nc.declare_dram_parameter("out", [ST, P, NWR, WW], f16, isOutput=True)

    with tile.TileContext(nc) as tc, ExitStack() as ctx:
        perm = ctx.enter_context(tc.tile_pool(name="perm", bufs=1))
        tmp = ctx.enter_context(tc.tile_pool(name="tmp", bufs=2))

        # R0: [dxi, t, 39, 37] (pad 1, dx in {-1,0,+1})
        # R1: [dxi, t, 43, 37] (pad 3, dx in {-3,0,+3})
        R0 = perm.tile([P, 3, ST, R0ROWS, W], f16, name="R0")
        R1 = perm.tile([P, 3, ST, R1ROWS, W], f16, name="R1")

        # ---- load fp16 x into the dx=0 plane centers ----
        for t in range(ST):
            nc.sync.dma_start(
                R0[:, 1, t, PAD0 : PAD0 + W, :].rearrange("p a b -> p (a b)"),
                xt.ap()[t],
            )

        for t in range(ST):
            # pad rows of the dx=0 plane (replicate first/last x row)
            nc.vector.tensor_copy(R0[:, 1, t, 0, :], R0[:, 1, t, 1, :])
            nc.vector.tensor_copy(R0[:, 1, t, R0ROWS - 1, :], R0[:, 1, t, R0ROWS - 2, :])

            # ---- dx=+-1 planes: flat-shifted whole-plane copies + col fix
            # (pad rows propagate; wrap errors land in the clamped column) ----
            p0f = R0[:, 1, t].rearrange("p a b -> p (a b)")
            f = R0[:, 0, t].rearrange("p a b -> p (a b)")
            nc.vector.tensor_copy(f[:, 1:R0F], p0f[:, 0 : R0F - 1])
            nc.vector.tensor_copy(R0[:, 0, t, :, 0], R0[:, 1, t, :, 0])
            f = R0[:, 2, t].rearrange("p a b -> p (a b)")
            nc.vector.tensor_copy(f[:, 0 : R0F - 1], p0f[:, 1:R0F])
            nc.vector.tensor_copy(R0[:, 2, t, :, W - 1], R0[:, 1, t, :, W - 1])

            # ---- k=0 stores for this ctile (one DMA per dy group) ----
            for r_i, dy in enumerate((-1, 0, 1)):
                src = R0[:, :, t, PAD0 + dy : PAD0 + dy + W, :].rearrange(
                    "p x a b -> p x (a b)"
                )
                nc.sync.dma_start(out.ap()[t][:, 3 * r_i : 3 * r_i + 3, :], src)

        for t in range(ST):
            Xc = R0[:, 1, t, PAD0 : PAD0 + W, :].rearrange("p a b -> p (a b)")
            Xc3 = R0[:, 1, t, PAD0 : PAD0 + W, :]

            # ---- column pass (fp16 sums):
            # T[i,j] = sum_dx X[i, j+dx] (zero outside) ----
            T = tmp.tile([P, WW], f16, name="T", tag="T")
            T3 = T.rearrange("p (a b) -> p a b", a=W, b=W)
            nc.vector.tensor_add(T[:, 0 : WW - 1], Xc[:, 0 : WW - 1], Xc[:, 1:WW])
            nc.vector.tensor_copy(T[:, WW - 1 : WW], Xc[:, WW - 1 : WW])
            nc.vector.tensor_add(T[:, 1:WW], T[:, 1:WW], Xc[:, 0 : WW - 1])
            nc.vector.tensor_add(T3[:, :, 0], Xc3[:, :, 0], Xc3[:, :, 1])
            nc.vector.tensor_add(T3[:, :, W - 1], Xc3[:, :, W - 2], Xc3[:, :, W - 1])

            # ---- row pass into a padded fp16 SUM plane ----
            Sp = tmp.tile([P, R1F], f16, name="Sp", tag="Sp")
            c0 = PAD1 * W                        # 111: center start
            nW = WW - W
            nc.vector.tensor_add(Sp[:, c0 : c0 + nW], T[:, 0:nW], T[:, W:WW])
            nc.vector.tensor_copy(Sp[:, c0 + nW : c0 + WW], T[:, nW:WW])
            nc.vector.tensor_add(Sp[:, c0 + W : c0 + WW], Sp[:, c0 + W : c0 + WW], T[:, 0:nW])
            # pad rows: replicate first/last center row (contiguous copies)
            for i in range(PAD1):
                nc.vector.tensor_copy(Sp[:, i * W : (i + 1) * W], Sp[:, c0 : c0 + W])
                nc.vector.tensor_copy(
                    Sp[:, (PAD1 + W + i) * W : (PAD1 + W + i + 1) * W],
                    Sp[:, (PAD1 + W - 1) * W : (PAD1 + W) * W],
                )

            # ---- normalize in ONE pass: interior count is 9; border
            # rows/cols have count 2 (not 3) per axis -> x1.5 ----
            Pc = R1[:, 1, t]
            Pcf = Pc.rearrange("p a b -> p (a b)")
            nc.vector.tensor_scalar_mul(Pcf[:, :], Sp[:, :], 1.0 / 9.0)
            nc.vector.tensor_scalar_mul(Pcf[:, 0 : c0 + W], Pcf[:, 0 : c0 + W], 1.5)
            nc.vector.tensor_scalar_mul(
                Pcf[:, R1F - c0 - W : R1F], Pcf[:, R1F - c0 - W : R1F], 1.5
            )
            nc.vector.tensor_scalar_mul(Pc[:, :, 0], Pc[:, :, 0], 1.5)
            nc.vector.tensor_scalar_mul(Pc[:, :, W - 1], Pc[:, :, W - 1], 1.5)

            # ---- dx=+-3 planes: flat-shifted fp16 copies + edge-col fixes ----
            f = R1[:, 0, t].rearrange("p a b -> p (a b)")
            nc.vector.tensor_copy(f[:, 3:R1F], Pcf[:, 0 : R1F - 3])
            f = R1[:, 2, t].rearrange("p a b -> p (a b)")
            nc.vector.tensor_copy(f[:, 0 : R1F - 3], Pcf[:, 3:R1F])
            for c in range(PAD1):
                nc.vector.tensor_copy(R1[:, 0, t, :, c], Pc[:, :, 0])
                nc.vector.tensor_copy(R1[:, 2, t, :, W - 1 - c], Pc[:, :, W - 1])

            # ---- k=1 stores: stage all 8 bins into ONE contiguous SBUF
            # block so the store DMA reads 21.9 KiB rows per partition
            # (fp16 2738B rows were measured row-rate limited ~340 GB/s) ----
            G1 = tmp.tile([P, 8, WW], f16, name="G1", tag="G1")
            pf = [R1[:, dxp, t].rearrange("p a b -> p (a b)") for dxp in range(3)]
            for s, (dy, dxp) in enumerate(
                ((-3, 0), (-3, 1), (-3, 2), (0, 0), (0, 2), (3, 0), (3, 1), (3, 2))
            ):
                lo = (PAD1 + dy) * W
                nc.vector.tensor_copy(G1[:, s, :], pf[dxp][:, lo : lo + WW])
            nc.sync.dma_start(out.ap()[t][:, 9:17, :], G1[:, :, :])

    nc.compile()
    return nc


def get_nc():
    if "nc" not in _CACHE:
        _CACHE["nc"] = _build_nc()
    return _CACHE["nc"]


def make_in_maps(x: np.ndarray):
    x = np.ascontiguousarray(x, dtype=np.float32)
    assert x.shape == (B, W, W, D), x.shape
    maps = []
    for b in range(B):
        xtr = x[b].transpose(2, 0, 1).reshape(ST, P, WW).astype(np.float16)
        maps.append({"xt": np.ascontiguousarray(xtr)})
    return maps


def decode_core(o, out):
    """Scatter the device 'out' tensor of one sample into out [NBINS*D, W, W]."""
    ob = np.asarray(o, dtype=np.float32)
    ob = ob.reshape(D, NWR, W, W).transpose(1, 0, 2, 3)
    out[: NWR * D] = ob.reshape(NWR * D, W, W)


def run(x: np.ndarray, **kw):
    from concourse.bass_utils import run_bass_kernel_spmd

    nc = get_nc()
    res = run_bass_kernel_spmd(nc, make_in_maps(x), core_ids=list(range(B)), **kw)
    outs = np.zeros((B, NBINS * D, W, W), np.float32)
    for b in range(B):
        decode_core(res.results[b]["out"], outs[b])
    return outs, res


def kernel(x: np.ndarray) -> np.ndarray:
    outs, _ = run(x)
    return outs


# revision 14
# speedup vs baseline: 1.1532x; 1.0142x over previous
"""DINOv2 LoRA featurizer histogram-binning kernel for TRN2 (8 NeuronCores).

Reference computation (per sample):
  x: [37, 37, 384] -> bx = x^T [384, 37, 37]
  pool0 = bx, pool1 = AvgPool2d(3, stride 1, pad 1, count_include_pad=False)
  17 bins = border-clamped shifts of pool0 (9 bins, offsets +-1) and
  pool1 (8 bins, offsets +-3); bins 17..28 of 29 are zero.
  out = [29*384, 37, 37] with channel c = bin*384 + feature.

Sharding: pure data parallel, sample b -> core b (B == 8 == n_cores).

The kernel is store-bandwidth bound (17 bins x 2.1 MB per core). Writing
fp16 (rel err ~4e-4, far inside the 2e-2 gate) halves the store stream;
only the 17 real bins live in DRAM and the 12 zero bins are assembled
host-side as zeros.

Device strategy (per core), built so the store-DMA stream (~18 MB fp16)
is the only critical path:
  - channels on partitions (3 tiles of 128), spatial flattened in free dim
  - x is uploaded as fp16 and DMA'd straight into the dx=0 pool0 plane,
    so k=0 bins are exactly fp16(x)
  - for each (pool k, dx) a column-pre-shifted, row-replicated-padded plane
    R[k][dxi][t]; every bin is then a CONTIGUOUS row-window of one plane.
    k=0 stores are grouped per (ctile, dy) — 3 adjacent bins per DMA so
    each partition writes an 8 KiB contiguous DRAM run; k=1 bins are
    staged into one contiguous SBUF block per ctile and stored as a
    single DMA with 21.9 KiB rows (fp16 2738B-row stores measured
    row-rate limited at ~340 GB/s)
  - everything rides the sync HWDGE ring: dual-ring alternation,
    cross-ctile merged DMAs, and deeper staging pools were all measured
    SLOWER; the gpsimd SWDGE ring is pathologically slow to drain —
    do not use it
  - dx!=0 planes are flat-shifted whole-plane fp16 copies (column shift ==
    element offset in the flat layout; row-boundary wrap errors land
    exactly in the clamped columns, fixed by strided edge-column copies)
  - pool1 separable 3x3 SUM runs in fp16 (2x DVE rate; |sums| <= ~20 so
    fp16 range is safe); count_include_pad=False normalization factorizes
    (cnt = rowcnt[i]*colcnt[j], each in {2,3}), so ONE tensor_scalar mul
    by 1/9 normalizes the plane and four tiny edge-region muls by 1.5 fix
    the border rows/columns
  - DVE does all element work (fp16 copies measured ~0.3 ns/elem; the Act
    engine has no fp16 speedup and mid-chain cross-engine handoffs cost
    more than they save)
  - no stride-0 (broadcast) APs, no GpSimd data ops (both measured slow)
"""

import numpy as np

B = 8
W = 37          # spatial side
WW = W * W      # 1369
D = 384
P = 128
ST = D // P     # 3 channel tiles of 128
NBINS = 29
NWR = 17        # bins actually written (the rest are zero)
PAD0, PAD1 = 1, 3
R0ROWS = W + 2 * PAD0             # 39
R1ROWS = W + 2 * PAD1             # 43
R0F = R0ROWS * W                  # 1443 flat elems per plane
R1F = R1ROWS * W                  # 1591

_CACHE = {}


def _build_nc():
    import concourse.bass as bass  # noqa: F401
    import concourse.tile as tile
    from concourse import bacc, mybir
    from contextlib import ExitStack

    f16 = mybir.dt.float16
    nc = bacc.Bacc("TRN2", target_bir_lowering=False, debug=False)

    xt = nc.declare_dram_parameter("xt", [ST, P, WW], f16, isOutput=False)
    out = nc.declare_dram_parameter("out", [ST, P, NWR, WW], f16, isOutput=True)

    with tile.TileContext(nc) as tc, ExitStack() as ctx:
        perm = ctx.enter_context(tc.tile_pool(name="perm", bufs=1))
        tmp = ctx.enter_context(tc.tile_pool(name="tmp", bufs=2))

        # R0: [dxi, t, 39, 37] (pad 1, dx in {-1,0,+1})
        # R1: [dxi, t, 43, 37] (pad 3, dx in {-3,0,+3})
        R0 = perm.tile([P, 3, ST, R0ROWS, W], f16, name="R0")
        R1 = perm.tile([P, 3, ST, R1ROWS, W], f16, name="R1")

        # ---- load fp16 x into the dx=0 plane centers ----
        for t in range(ST):
            nc.sync.dma_start(
                R0[:, 1, t, PAD0 : PAD0 + W, :].rearrange("p a b -> p (a b)"),
                xt.ap()[t],
            )

        for t in range(ST):
            # pad rows of the dx=0 plane (replicate first/last x row)
            nc.vector.tensor_copy(R0[:, 1, t, 0, :], R0[:, 1, t, 1, :])
            nc.vector.tensor_copy(R0[:, 1, t, R0ROWS - 1, :], R0[:, 1, t, R0ROWS - 2, :])

            # ---- dx=+-1 planes: flat-shifted whole-plane copies + col fix
            # (pad rows propagate; wrap errors land in the clamped column) ----
            p0f = R0[:, 1, t].rearrange("p a b -> p (a b)")
            f = R0[:, 0, t].rearrange("p a b -> p (a b)")
            nc.vector.tensor_copy(f[:, 1:R0F], p0f[:, 0 : R0F - 1])
            nc.vector.tensor_copy(R0[:, 0, t, :, 0], R0[:, 1, t, :, 0])
            f = R0[:, 2, t].rearrange("p a b -> p (a b)")
            nc.vector.tensor_copy(f[:, 0 : R0F - 1], p0f[:, 1:R0F])
            nc.vector.tensor_copy(R0[:, 2, t, :, W - 1], R0[:, 1, t, :, W - 1])

            # ---- k=0 stores for this ctile (one DMA per dy group) ----
            for r_i, dy in enumerate((-1, 0, 1)):
                src = R0[:, :, t, PAD0 + dy : PAD0 + dy + W, :].rearrange(
                    "p x a b -> p x (a b)"
                )
                nc.sync.dma_start(out.ap()[t][:, 3 * r_i : 3 * r_i + 3, :], src)

        for t in range(ST):
            Xc = R0[:, 1, t, PAD0 : PAD0 + W, :].rearrange("p a b -> p (a b)")
            Xc3 = R0[:, 1, t, PAD0 : PAD0 + W, :]

            # ---- column pass (fp16 sums):
            # T[i,j] = sum_dx X[i, j+dx] (zero outside) ----
            T = tmp.tile([P, WW], f16, name="T", tag="T")
            T3 = T.rearrange("p (a b) -> p a b", a=W, b=W)
            nc.vector.tensor_add(T[:, 0 : WW - 1], Xc[:, 0 : WW - 1], Xc[:, 1:WW])
            nc.vector.tensor_copy(T[:, WW - 1 : WW], Xc[:, WW - 1 : WW])
            nc.vector.tensor_add(T[:, 1:WW], T[:, 1:WW], Xc[:, 0 : WW - 1])
            nc.vector.tensor_add(T3[:, :, 0], Xc3[:, :, 0], Xc3[:, :, 1])
            nc.vector.tensor_add(T3[:, :, W - 1], Xc3[:, :, W - 2], Xc3[:, :, W - 1])

            # ---- row pass into a padded fp16 SUM plane ----
            Sp = tmp.tile([P, R1F], f16, name="Sp", tag="Sp")
            c0 = PAD1 * W                        # 111: center start
            nW = WW - W
            nc.vector.tensor_add(Sp[:, c0 : c0 + nW], T[:, 0:nW], T[:, W:WW])
            nc.vector.tensor_copy(Sp[:, c0 + nW : c0 + WW], T[:, nW:WW])
            nc.vector.tensor_add(Sp[:, c0 + W : c0 + WW], Sp[:, c0 + W : c0 + WW], T[:, 0:nW])
            # pad rows: replicate first/last center row (contiguous copies)
            for i in range(PAD1):
                nc.vector.tensor_copy(Sp[:, i * W : (i + 1) * W], Sp[:, c0 : c0 + W])
                nc.vector.tensor_copy(
                    Sp[:, (PAD1 + W + i) * W : (PAD1 + W + i + 1) * W],
                    Sp[:, (PAD1 + W - 1) * W : (PAD1 + W) * W],
                )

            # ---- normalize in ONE pass: interior count is 9; border
            # rows/cols have count 2 (not 3) per axis -> x1.5 ----
            Pc = R1[:, 1, t]
            Pcf = Pc.rearrange("p a b -> p (a b)")
            nc.vector.tensor_scalar_mul(Pcf[:, :], Sp[:, :], 1.0 / 9.0)
            nc.vector.tensor_scalar_mul(Pcf[:, 0 : c0 + W], Pcf[:, 0 : c0 + W], 1.5)
            nc.vector.tensor_scalar_mul(
                Pcf[:, R1F - c0 - W : R1F], Pcf[:, R1F - c0 - W : R1F], 1.5
            )
            nc.vector.tensor_scalar_mul(Pc[:, :, 0], Pc[:, :, 0], 1.5)
            nc.vector.tensor_scalar_mul(Pc[:, :, W - 1], Pc[:, :, W - 1], 1.5)

            # ---- dx=+-3 planes: flat-shifted fp16 copies + edge-col fixes ----
            f = R1[:, 0, t].rearrange("p a b -> p (a b)")
            nc.vector.tensor_copy(f[:, 3:R1F], Pcf[:, 0 : R1F - 3])
            f = R1[:, 2, t].rearrange("p a b -> p (a b)")
            nc.vector.tensor_copy(f[:, 0 : R1F - 3], Pcf[:, 3:R1F])
            for c in range(PAD1):
                nc.vector.tensor_copy(R1[:, 0, t, :, c], Pc[:, :, 0])
                nc.vector.tensor_copy(R1[:, 2, t, :, W - 1 - c], Pc[:, :, W - 1])

            # ---- k=1 stores: stage all 8 bins into ONE contiguous SBUF
            # block so the store DMA reads 21.9 KiB rows per partition
            # (fp16 2738B rows were measured row-rate limited ~340 GB/s) ----
            G1 = tmp.tile([P, 8, WW], f16, name="G1", tag="G1")
            pf = [R1[:, dxp, t].rearrange("p a b -> p (a b)") for dxp in range(3)]
            for s, (dy, dxp) in enumerate(
                ((-3, 0), (-3, 1), (-3, 2), (0, 0), (0, 2), (3, 0), (3, 1), (3, 2))
            ):
                lo = (PAD1 + dy) * W
                nc.vector.tensor_copy(G1[:, s, :], pf[dxp][:, lo : lo + WW])
            # split into k0-sized pieces (~1 MB): measured ~430 GB/s vs
            # ~310 for one big 2.9 MB DMA; the small piece goes last so the
            # final transfer tail is short
            nc.sync.dma_start(out.ap()[t][:, 9:12, :], G1[:, 0:3, :])
            nc.sync.dma_start(out.ap()[t][:, 12:15, :], G1[:, 3:6, :])
            nc.sync.dma_start(out.ap()[t][:, 15:17, :], G1[:, 6:8, :])

    nc.compile()
    return nc


def get_nc():
    if "nc" not in _CACHE:
        _CACHE["nc"] = _build_nc()
    return _CACHE["nc"]


def make_in_maps(x: np.ndarray):
    x = np.ascontiguousarray(x, dtype=np.float32)
    assert x.shape == (B, W, W, D), x.shape
    maps = []
    for b in range(B):
        xtr = x[b].transpose(2, 0, 1).reshape(ST, P, WW).astype(np.float16)
        maps.append({"xt": np.ascontiguousarray(xtr)})
    return maps


def decode_core(o, out):
    """Scatter the device 'out' tensor of one sample into out [NBINS*D, W, W]."""
    ob = np.asarray(o, dtype=np.float32)
    ob = ob.reshape(D, NWR, W, W).transpose(1, 0, 2, 3)
    out[: NWR * D] = ob.reshape(NWR * D, W, W)


def run(x: np.ndarray, **kw):
    from concourse.bass_utils import run_bass_kernel_spmd

    nc = get_nc()
    res = run_bass_kernel_spmd(nc, make_in_maps(x), core_ids=list(range(B)), **kw)
    outs = np.zeros((B, NBINS * D, W, W), np.float32)
    for b in range(B):
        decode_core(res.results[b]["out"], outs[b])
    return outs, res


def kernel(x: np.ndarray) -> np.ndarray:
    outs, _ = run(x)
    return outs
